# revision 22
# baseline (speedup 1.0000x reference)
# Contrastive loss (CLIP-style) on 8 Trainium2 NeuronCores.
#
# reference:
#   img = l2norm(image_embeds); txt = l2norm(text_embeds)        # [N, D]
#   sim = img @ txt.T                                            # [N, N]
#   loss = mean(logsumexp(sim - 1, axis=-1) - diag(sim))
#
# Distribution: shard both embedding batches along N across the 8 cores.
# Per core:
#   1. Normalize own txt block (fp32 stats via fused tensor_tensor_reduce),
#      emit 8*txt_n as bf16 (gpsimd), PE-transpose to [d, n] layout, cast to
#      fp8e4 (scalar), write to DRAM in two column halves; each half is
#      AllGathered separately so the GEMM can start on half 1 while half 2
#      is still in flight.
#   2. Own img block stays RAW: cast 16*img to bf16, PE-transpose, cast fp8
#      into SBUF-resident lhsT [p, k, m]. The 1/||img_i|| factor is folded
#      into the exp() activation's per-partition scale (exp(psum*inv_i/128)).
#   3. Positives computed exactly as fp32 row-dots of the raw blocks.
#   4. Main loop: per (half, block), one bulk fp8 DMA, then fp8 DoubleRow
#      matmuls (256-deep contraction per instruction) accumulate [128, 512]
#      fp32 PSUM tiles; ScalarE exp with accum_out produces per-row partial
#      sums; final ln + subtract positives per row.
# Host gathers the 8 x [P, mt] row values and returns mean - margin
# (logsumexp(sim - 1) == logsumexp(sim) - 1; |sim| <= 1.1 so no row-max pass).

import os

import numpy as np

N_TOTAL = 8192
D_FULL = 1024
N_CORES = 8
P = 128
NCHUNK = 512
MARGIN = 1.0
IMG_SCALE = 16.0  # raw img premultiplier before fp8 cast
TXT_SCALE = 8.0   # normalized txt premultiplier before fp8 cast

LAST_EXEC_NS = None
LAST_PROFILE = None


def build_bass(n_total=N_TOTAL, d=D_FULL, n_cores=N_CORES, nchunk=NCHUNK):
    import concourse.mybir as mybir
    import concourse.tile as tile
    from concourse import bacc, masks
    from concourse.bass import ts as bass_ts

    dt = mybir.dt
    Alu = mybir.AluOpType
    Act = mybir.ActivationFunctionType
    AxisX = mybir.AxisListType.X
    DR = mybir.MatmulPerfMode.DoubleRow

    blk = n_total // n_cores     # 1024 local rows
    kt = d // P                  # 8 contraction tiles of 128
    kp = kt // 2                 # 4 DoubleRow k-pairs
    mt = blk // P                # 8 local img row tiles
    hn = blk // nchunk           # 2 column halves per block
    n_chunks = n_cores * hn      # 16 psum chunks per row tile
    th = mt // hn                # txt tiles per column half (4)
    assert blk % P == 0 and d % P == 0 and kt % 2 == 0 and blk % nchunk == 0

    nc = bacc.Bacc(
        "TRN2", target_bir_lowering=False, debug=False, num_devices=n_cores
    )
    img = nc.dram_tensor("img_block", [blk, d], dt.float32, kind="ExternalInput")
    txt = nc.dram_tensor("txt_block", [blk, d], dt.float32, kind="ExternalInput")
    out = nc.dram_tensor("out_rows", [P, mt], dt.float32, kind="ExternalOutput")

    with tile.TileContext(nc) as tc:
        with (
            tc.tile_pool(name="dram", bufs=1, space="DRAM") as dram_pool,
            tc.tile_pool(name="persist", bufs=1) as persist,
            tc.tile_pool(name="txtf32", bufs=1) as txtf32,
            tc.tile_pool(name="imf32", bufs=1) as imf32,
            tc.tile_pool(name="scratch", bufs=2) as scratch,
            tc.tile_pool(name="bfp", bufs=2) as bfp,
            tc.tile_pool(name="small", bufs=2) as small,
            tc.tile_pool(name="rhsp", bufs=3) as rhsp,
            tc.tile_pool(name="expp", bufs=4) as expp,
            tc.tile_pool(name="psum_t", bufs=2, space="PSUM") as psum_t,
            tc.tile_pool(name="psum_g", bufs=4, space="PSUM") as psum_g,
        ):
            # txtT_dram_h[h] index (p, k, n): d-dim k*P+p, local col h*512+n
            txtT_h = [
                dram_pool.tile([P, kt, nchunk], dt.float8e4, name=f"txtT_h{h}")
                for h in range(hn)
            ]
            agT_h = [
                dram_pool.tile(
                    [n_cores * P, kt, nchunk], dt.float8e4, name=f"agT_h{h}",
                    addr_space="Shared",
                )
                for h in range(hn)
            ]

            ident = persist.tile([P, P], dt.bfloat16, name="ident")
            masks.make_identity(nc, ident[:])

            imgT_sb = persist.tile([P, kt, blk], dt.float8e4, name="imgT_sb")
            txtT_sb = persist.tile([P, kt, blk], dt.float8e4, name="txtT_sb")
            exp_scale = persist.tile([P, mt], dt.float32, name="exp_scale")
            pos_all = persist.tile([P, mt], dt.float32, name="pos_all")
            sums_all = persist.tile([P, mt * n_chunks], dt.float32, name="sums_all")
            out_all = persist.tile([P, mt], dt.float32, name="out_all")

            # ---- issue all input loads up front (txt first: it gates the AG) --
            txt_nat, img_nat = [], []
            for t in range(mt):
                tn = txtf32.tile([P, d], dt.float32, name=f"txtnat{t}")
                nc.sync.dma_start(tn[:], txt[t * P : (t + 1) * P, :])
                txt_nat.append(tn)
            for t in range(mt):
                im = imf32.tile([P, d], dt.float32, name=f"imnat{t}")
                nc.sync.dma_start(im[:], img[t * P : (t + 1) * P, :])
                img_nat.append(im)

            invt = []

            def txt_tile(t):
                # fused square + row-sum
                sq = scratch.tile([P, d], dt.float32, name="sq", tag="sq")
                nc.vector.tensor_mul(sq[:], txt_nat[t][:], txt_nat[t][:])
                n2 = small.tile([P, 1], dt.float32, name="n2", tag="n2")
                nc.vector.reduce_sum(n2[:], sq[:], axis=AxisX)
                r2 = small.tile([P, 1], dt.float32, name="r2", tag="r2")
                nc.vector.reciprocal(r2[:], n2[:])
                iv = persist.tile([P, 1], dt.float32, name=f"invt{t}")
                # 8/||t|| = sqrt(64 / ||t||^2)
                nc.scalar.activation(iv[:], r2[:], Act.Sqrt, scale=TXT_SCALE * TXT_SCALE)
                invt.append(iv)
                # 8 * normalized txt -> bf16 (gpsimd, SBUF->SBUF)
                tb = bfp.tile([P, d], dt.bfloat16, name="txtbf", tag="txtbf")
                nc.scalar.mul(tb[:], txt_nat[t][:], iv[:])
                # PE transpose + fp8 cast into txtT_sb[:, :, t*P:(t+1)*P]
                for half in range(2):
                    pst = psum_t.tile([P, 4, P], dt.bfloat16, name="pst", tag="pst")
                    for j in range(4):
                        k = half * 4 + j
                        nc.tensor.matmul(
                            pst[:, j, :],
                            lhsT=tb[:, k * P : (k + 1) * P],
                            rhs=ident[:],
                            is_transpose=True,
                        )
                    nc.scalar.copy(
                        txtT_sb[:, half * 4 : (half + 1) * 4, t * P : (t + 1) * P],
                        pst[:],
                    )

            # ---- txt halves -> DMA out -> AllGather per half ----
            for h in range(hn):
                for t in range(h * th, (h + 1) * th):
                    txt_tile(t)
                for k in range(kt):
                    nc.sync.dma_start(
                        txtT_h[h][:, k, :],
                        txtT_sb[:, k, h * nchunk : (h + 1) * nchunk],
                    )
                nc.gpsimd.collective_compute(
                    "AllGather",
                    Alu.bypass,
                    replica_groups=[list(range(n_cores))],
                    ins=[txtT_h[h].opt()],
                    outs=[agT_h[h].opt()],
                )

            # ---- img pass (overlaps AG): stats, exp scale, positives, lhsT ----
            for t in range(mt):
                im = img_nat[t]
                sq = scratch.tile([P, d], dt.float32, name="sqi", tag="sqi")
                nc.vector.tensor_mul(sq[:], im[:], im[:])
                n2 = small.tile([P, 1], dt.float32, name="n2i", tag="n2i")
                nc.vector.reduce_sum(n2[:], sq[:], axis=AxisX)
                r2 = small.tile([P, 1], dt.float32, name="r2i", tag="r2i")
                nc.vector.reciprocal(r2[:], n2[:])
                iv16 = small.tile([P, 1], dt.float32, name="iv16", tag="iv16")
                nc.scalar.activation(
                    iv16[:], r2[:], Act.Sqrt, scale=IMG_SCALE * IMG_SCALE
                )
                # exp scale: 1/(128*||i||) = sqrt(r2/16384)
                nc.scalar.activation(
                    exp_scale[:, t : t + 1], r2[:], Act.Sqrt,
                    scale=1.0 / (IMG_SCALE * IMG_SCALE * TXT_SCALE * TXT_SCALE),
                )

                # positives: dot(raw_i, raw_t) * inv_i * inv_t (fused dot)
                pr = scratch.tile([P, d], dt.float32, name="pr", tag="pr")
                nc.vector.tensor_mul(pr[:], im[:], txt_nat[t][:])
                dv = small.tile([P, 1], dt.float32, name="dv", tag="dv")
                nc.vector.reduce_sum(dv[:], pr[:], axis=AxisX)
                pp = small.tile([P, 1], dt.float32, name="pp", tag="pp")
                nc.vector.tensor_mul(pp[:], iv16[:], invt[t][:])  # 128*invi*invt
                pq = small.tile([P, 1], dt.float32, name="pq", tag="pq")
                nc.vector.tensor_scalar_mul(pq[:], dv[:], pp[:])
                nc.vector.tensor_scalar_mul(
                    pos_all[:, t : t + 1], pq[:], 1.0 / (IMG_SCALE * TXT_SCALE)
                )

                ib = bfp.tile([P, d], dt.bfloat16, name="imgbf", tag="imgbf")
                nc.scalar.mul(ib[:], im[:], IMG_SCALE)
                for half in range(2):
                    psi = psum_t.tile([P, 4, P], dt.bfloat16, name="psi", tag="psi")
                    for j in range(4):
                        k = half * 4 + j
                        nc.tensor.matmul(
                            psi[:, j, :],
                            lhsT=ib[:, k * P : (k + 1) * P],
                            rhs=ident[:],
                            is_transpose=True,
                        )
                    nc.scalar.copy(
                        imgT_sb[:, half * 4 : (half + 1) * 4, t * P : (t + 1) * P],
                        psi[:],
                    )

            # ---- main loop: fp8 DoubleRow GEMM + exp row-sums ----
            def gemm_chunk(m, rhs_ap, idx):
                ps = psum_g.tile([P, nchunk], dt.float32, name="ps", tag="ps")
                for K in range(kp):
                    nc.tensor.matmul(
                        ps[:],
                        lhsT=imgT_sb[:, 2 * K : 2 * K + 2, m * P : (m + 1) * P],
                        rhs=rhs_ap[:, 2 * K : 2 * K + 2, :],
                        start=(K == 0),
                        stop=(K == kp - 1),
                        perf_mode=DR,
                    )
                ex = expp.tile([P, nchunk], dt.bfloat16, name="ex", tag="ex")
                nc.scalar.activation(
                    ex[:], ps[:], Act.Exp,
                    scale=exp_scale[:, m : m + 1],
                    accum_out=sums_all[:, idx : idx + 1],
                )

            # own block straight from SBUF -- no AllGather dependency
            for m in range(mt):
                for h in range(hn):
                    gemm_chunk(
                        m,
                        txtT_sb[:, :, h * nchunk : (h + 1) * nchunk],
                        m * n_chunks + h,
                    )

            # remote blocks (pid+1 .. pid+7) from the gathered text
            pid = nc.sync.partition_id()
            for h in range(hn):
                for i in range(1, n_cores):
                    slot = (pid + i) % n_cores
                    rt = rhsp.tile([P, kt, nchunk], dt.float8e4, name="rt", tag="rt")
                    nc.sync.dma_start(rt[:], agT_h[h][bass_ts(slot, P), :, :])
                    for m in range(mt):
                        gemm_chunk(
                            m, rt, m * n_chunks + hn + h * (n_cores - 1) + (i - 1)
                        )

            # ---- tail: lse - positives, batched across all m ----
            rs = small.tile([P, mt], dt.float32, name="rs", tag="rs")
            nc.vector.reduce_sum(
                rs[:], sums_all[:].rearrange("p (m c) -> p m c", c=n_chunks),
                axis=AxisX,
            )
            lse = small.tile([P, mt], dt.float32, name="lse", tag="lse")
            nc.scalar.activation(lse[:], rs[:], Act.Ln)
            nc.vector.tensor_sub(out_all[:], lse[:], pos_all[:])

            nc.sync.dma_start(out.ap(), out_all[:])

    nc.compile()
    return nc


_NC_CACHE = {}


def _get_nc(key=(N_TOTAL, D_FULL, N_CORES, NCHUNK)):
    if key not in _NC_CACHE:
        _NC_CACHE[key] = build_bass(*key)
    return _NC_CACHE[key]


def kernel(image_embeds: np.ndarray, text_embeds: np.ndarray) -> np.ndarray:
    global LAST_EXEC_NS, LAST_PROFILE
    from concourse import bass_utils

    image_embeds = np.ascontiguousarray(np.asarray(image_embeds, dtype=np.float32))
    text_embeds = np.ascontiguousarray(np.asarray(text_embeds, dtype=np.float32))
    assert image_embeds.shape == (N_TOTAL, D_FULL)
    assert text_embeds.shape == (N_TOTAL, D_FULL)

    nc = _get_nc()
    blk = N_TOTAL // N_CORES
    in_maps = [
        {
            "img_block": np.ascontiguousarray(image_embeds[c * blk : (c + 1) * blk]),
            "txt_block": np.ascontiguousarray(text_embeds[c * blk : (c + 1) * blk]),
        }
        for c in range(N_CORES)
    ]
    trace = os.environ.get("KERNEL_TRACE", "0") == "1"
    res = bass_utils.run_bass_kernel_spmd(
        nc, in_maps, core_ids=list(range(N_CORES)), trace=trace
    )
    LAST_EXEC_NS = res.exec_time_ns
    LAST_PROFILE = res.profile_json
    globals()["LAST_RESULT"] = res

    mt = blk // P
    rows = []
    for c in range(N_CORES):
        o = np.asarray(res.results[c]["out_rows"])  # [P, mt]
        rows.append(o.T.reshape(-1))  # local row i = m*P + p
    vals = np.concatenate(rows)  # [N_TOTAL]
    result = np.float32(np.mean(vals.astype(np.float64)) - MARGIN)
    return np.asarray(result, dtype=np.float32)


# revision 23
# speedup vs baseline: 1.0066x; 1.0066x over previous
# Contrastive loss (CLIP-style) on 8 Trainium2 NeuronCores.
#
# reference:
#   img = l2norm(image_embeds); txt = l2norm(text_embeds)        # [N, D]
#   sim = img @ txt.T                                            # [N, N]
#   loss = mean(logsumexp(sim - 1, axis=-1) - diag(sim))
#
# Distribution: shard both embedding batches along N across the 8 cores.
# Per core:
#   1. Normalize own txt block (fp32 stats via fused tensor_tensor_reduce),
#      emit 8*txt_n as bf16 (gpsimd), PE-transpose to [d, n] layout, cast to
#      fp8e4 (scalar), write to DRAM in two column halves; each half is
#      AllGathered separately so the GEMM can start on half 1 while half 2
#      is still in flight.
#   2. Own img block stays RAW: cast 16*img to bf16, PE-transpose, cast fp8
#      into SBUF-resident lhsT [p, k, m]. The 1/||img_i|| factor is folded
#      into the exp() activation's per-partition scale (exp(psum*inv_i/128)).
#   3. Positives computed exactly as fp32 row-dots of the raw blocks.
#   4. Main loop: per (half, block), one bulk fp8 DMA, then fp8 DoubleRow
#      matmuls (256-deep contraction per instruction) accumulate [128, 512]
#      fp32 PSUM tiles; ScalarE exp with accum_out produces per-row partial
#      sums; final ln + subtract positives per row.
# Host gathers the 8 x [P, mt] row values and returns mean - margin
# (logsumexp(sim - 1) == logsumexp(sim) - 1; |sim| <= 1.1 so no row-max pass).

import os

import numpy as np

N_TOTAL = 8192
D_FULL = 1024
N_CORES = 8
P = 128
NCHUNK = 512
MARGIN = 1.0
IMG_SCALE = 16.0  # raw img premultiplier before fp8 cast
TXT_SCALE = 8.0   # normalized txt premultiplier before fp8 cast

LAST_EXEC_NS = None
LAST_PROFILE = None


def build_bass(n_total=N_TOTAL, d=D_FULL, n_cores=N_CORES, nchunk=NCHUNK):
    import concourse.mybir as mybir
    import concourse.tile as tile
    from concourse import bacc, masks
    from concourse.bass import ts as bass_ts

    dt = mybir.dt
    Alu = mybir.AluOpType
    Act = mybir.ActivationFunctionType
    AxisX = mybir.AxisListType.X
    DR = mybir.MatmulPerfMode.DoubleRow

    blk = n_total // n_cores     # 1024 local rows
    kt = d // P                  # 8 contraction tiles of 128
    kp = kt // 2                 # 4 DoubleRow k-pairs
    mt = blk // P                # 8 local img row tiles
    hn = blk // nchunk           # 2 column halves per block
    n_chunks = n_cores * hn      # 16 psum chunks per row tile
    th = mt // hn                # txt tiles per column half (4)
    assert blk % P == 0 and d % P == 0 and kt % 2 == 0 and blk % nchunk == 0

    nc = bacc.Bacc(
        "TRN2", target_bir_lowering=False, debug=False, num_devices=n_cores
    )
    img = nc.dram_tensor("img_block", [blk, d], dt.float32, kind="ExternalInput")
    txt = nc.dram_tensor("txt_block", [blk, d], dt.float32, kind="ExternalInput")
    out = nc.dram_tensor("out_rows", [P, mt], dt.float32, kind="ExternalOutput")

    with tile.TileContext(nc) as tc:
        with (
            tc.tile_pool(name="dram", bufs=1, space="DRAM") as dram_pool,
            tc.tile_pool(name="persist", bufs=1) as persist,
            tc.tile_pool(name="txtf32", bufs=1) as txtf32,
            tc.tile_pool(name="imf32", bufs=1) as imf32,
            tc.tile_pool(name="scratch", bufs=2) as scratch,
            tc.tile_pool(name="bfp", bufs=2) as bfp,
            tc.tile_pool(name="small", bufs=2) as small,
            tc.tile_pool(name="rhsp", bufs=3) as rhsp,
            tc.tile_pool(name="expp", bufs=4) as expp,
            tc.tile_pool(name="psum_t", bufs=1, space="PSUM") as psum_t,
            tc.tile_pool(name="psum_g", bufs=6, space="PSUM") as psum_g,
        ):
            # txtT_dram_h[h] index (p, k, n): d-dim k*P+p, local col h*512+n
            txtT_h = [
                dram_pool.tile([P, kt, nchunk], dt.float8e4, name=f"txtT_h{h}")
                for h in range(hn)
            ]
            agT_h = [
                dram_pool.tile(
                    [n_cores * P, kt, nchunk], dt.float8e4, name=f"agT_h{h}",
                    addr_space="Shared",
                )
                for h in range(hn)
            ]

            ident = persist.tile([P, P], dt.bfloat16, name="ident")
            masks.make_identity(nc, ident[:])

            imgT_sb = persist.tile([P, kt, blk], dt.float8e4, name="imgT_sb")
            txtT_sb = persist.tile([P, kt, blk], dt.float8e4, name="txtT_sb")
            exp_scale = persist.tile([P, mt], dt.float32, name="exp_scale")
            pos_all = persist.tile([P, mt], dt.float32, name="pos_all")
            sums_all = persist.tile([P, mt * n_chunks], dt.float32, name="sums_all")
            out_all = persist.tile([P, mt], dt.float32, name="out_all")

            # ---- issue all input loads up front (txt first: it gates the AG) --
            txt_nat, img_nat = [], []
            for t in range(mt):
                tn = txtf32.tile([P, d], dt.float32, name=f"txtnat{t}")
                nc.sync.dma_start(tn[:], txt[t * P : (t + 1) * P, :])
                txt_nat.append(tn)
            for t in range(mt):
                im = imf32.tile([P, d], dt.float32, name=f"imnat{t}")
                nc.sync.dma_start(im[:], img[t * P : (t + 1) * P, :])
                img_nat.append(im)

            invt = []

            def txt_tile(t):
                # fused square + row-sum
                sq = scratch.tile([P, d], dt.float32, name="sq", tag="sq")
                nc.vector.tensor_mul(sq[:], txt_nat[t][:], txt_nat[t][:])
                n2 = small.tile([P, 1], dt.float32, name="n2", tag="n2")
                nc.vector.reduce_sum(n2[:], sq[:], axis=AxisX)
                r2 = small.tile([P, 1], dt.float32, name="r2", tag="r2")
                nc.vector.reciprocal(r2[:], n2[:])
                iv = persist.tile([P, 1], dt.float32, name=f"invt{t}")
                # 8/||t|| = sqrt(64 / ||t||^2)
                nc.scalar.activation(iv[:], r2[:], Act.Sqrt, scale=TXT_SCALE * TXT_SCALE)
                invt.append(iv)
                # 8 * normalized txt -> bf16 (gpsimd, SBUF->SBUF)
                tb = bfp.tile([P, d], dt.bfloat16, name="txtbf", tag="txtbf")
                nc.scalar.mul(tb[:], txt_nat[t][:], iv[:])
                # PE transpose + fp8 cast into txtT_sb[:, :, t*P:(t+1)*P]
                pst = psum_t.tile([P, kt, P], dt.bfloat16, name="pst", tag="pst")
                for k in range(kt):
                    nc.tensor.matmul(
                        pst[:, k, :],
                        lhsT=tb[:, k * P : (k + 1) * P],
                        rhs=ident[:],
                        is_transpose=True,
                    )
                nc.scalar.copy(
                    txtT_sb[:, :, t * P : (t + 1) * P], pst[:]
                )

            # ---- txt halves -> DMA out -> AllGather per half ----
            for h in range(hn):
                for t in range(h * th, (h + 1) * th):
                    txt_tile(t)
                for k in range(kt):
                    nc.sync.dma_start(
                        txtT_h[h][:, k, :],
                        txtT_sb[:, k, h * nchunk : (h + 1) * nchunk],
                    )
                nc.gpsimd.collective_compute(
                    "AllGather",
                    Alu.bypass,
                    replica_groups=[list(range(n_cores))],
                    ins=[txtT_h[h].opt()],
                    outs=[agT_h[h].opt()],
                )

            # ---- img pass (overlaps AG): stats, exp scale, positives, lhsT ----
            for t in range(mt):
                im = img_nat[t]
                sq = scratch.tile([P, d], dt.float32, name="sqi", tag="sqi")
                nc.vector.tensor_mul(sq[:], im[:], im[:])
                n2 = small.tile([P, 1], dt.float32, name="n2i", tag="n2i")
                nc.vector.reduce_sum(n2[:], sq[:], axis=AxisX)
                r2 = small.tile([P, 1], dt.float32, name="r2i", tag="r2i")
                nc.vector.reciprocal(r2[:], n2[:])
                iv16 = small.tile([P, 1], dt.float32, name="iv16", tag="iv16")
                nc.scalar.activation(
                    iv16[:], r2[:], Act.Sqrt, scale=IMG_SCALE * IMG_SCALE
                )
                # exp scale: 1/(128*||i||) = sqrt(r2/16384)
                nc.scalar.activation(
                    exp_scale[:, t : t + 1], r2[:], Act.Sqrt,
                    scale=1.0 / (IMG_SCALE * IMG_SCALE * TXT_SCALE * TXT_SCALE),
                )

                # positives: dot(raw_i, raw_t) * inv_i * inv_t (fused dot)
                pr = scratch.tile([P, d], dt.float32, name="pr", tag="pr")
                nc.vector.tensor_mul(pr[:], im[:], txt_nat[t][:])
                dv = small.tile([P, 1], dt.float32, name="dv", tag="dv")
                nc.vector.reduce_sum(dv[:], pr[:], axis=AxisX)
                pp = small.tile([P, 1], dt.float32, name="pp", tag="pp")
                nc.vector.tensor_mul(pp[:], iv16[:], invt[t][:])  # 128*invi*invt
                pq = small.tile([P, 1], dt.float32, name="pq", tag="pq")
                nc.vector.tensor_scalar_mul(pq[:], dv[:], pp[:])
                nc.vector.tensor_scalar_mul(
                    pos_all[:, t : t + 1], pq[:], 1.0 / (IMG_SCALE * TXT_SCALE)
                )

                ib = bfp.tile([P, d], dt.bfloat16, name="imgbf", tag="imgbf")
                nc.scalar.mul(ib[:], im[:], IMG_SCALE)
                psi = psum_t.tile([P, kt, P], dt.bfloat16, name="psi", tag="psi")
                for k in range(kt):
                    nc.tensor.matmul(
                        psi[:, k, :],
                        lhsT=ib[:, k * P : (k + 1) * P],
                        rhs=ident[:],
                        is_transpose=True,
                    )
                nc.scalar.copy(
                    imgT_sb[:, :, t * P : (t + 1) * P], psi[:]
                )

            # ---- main loop: fp8 DoubleRow GEMM + exp row-sums ----
            def gemm_chunk(m, rhs_ap, idx):
                ps = psum_g.tile([P, nchunk], dt.float32, name="ps", tag="ps")
                for K in range(kp):
                    nc.tensor.matmul(
                        ps[:],
                        lhsT=imgT_sb[:, 2 * K : 2 * K + 2, m * P : (m + 1) * P],
                        rhs=rhs_ap[:, 2 * K : 2 * K + 2, :],
                        start=(K == 0),
                        stop=(K == kp - 1),
                        perf_mode=DR,
                    )
                ex = expp.tile([P, nchunk], dt.bfloat16, name="ex", tag="ex")
                nc.scalar.activation(
                    ex[:], ps[:], Act.Exp,
                    scale=exp_scale[:, m : m + 1],
                    accum_out=sums_all[:, idx : idx + 1],
                )

            # own block straight from SBUF -- no AllGather dependency
            for m in range(mt):
                for h in range(hn):
                    gemm_chunk(
                        m,
                        txtT_sb[:, :, h * nchunk : (h + 1) * nchunk],
                        m * n_chunks + h,
                    )

            # remote blocks (pid+1 .. pid+7) from the gathered text
            pid = nc.sync.partition_id()
            for h in range(hn):
                for i in range(1, n_cores):
                    slot = (pid + i) % n_cores
                    rt = rhsp.tile([P, kt, nchunk], dt.float8e4, name="rt", tag="rt")
                    nc.sync.dma_start(rt[:], agT_h[h][bass_ts(slot, P), :, :])
                    for m in range(mt):
                        gemm_chunk(
                            m, rt, m * n_chunks + hn + h * (n_cores - 1) + (i - 1)
                        )

            # ---- tail: lse - positives, batched across all m ----
            rs = small.tile([P, mt], dt.float32, name="rs", tag="rs")
            nc.vector.reduce_sum(
                rs[:], sums_all[:].rearrange("p (m c) -> p m c", c=n_chunks),
                axis=AxisX,
            )
            lse = small.tile([P, mt], dt.float32, name="lse", tag="lse")
            nc.scalar.activation(lse[:], rs[:], Act.Ln)
            nc.vector.tensor_sub(out_all[:], lse[:], pos_all[:])

            nc.sync.dma_start(out.ap(), out_all[:])

    nc.compile()
    return nc


_NC_CACHE = {}


def _get_nc(key=(N_TOTAL, D_FULL, N_CORES, NCHUNK)):
    if key not in _NC_CACHE:
        _NC_CACHE[key] = build_bass(*key)
    return _NC_CACHE[key]


def kernel(image_embeds: np.ndarray, text_embeds: np.ndarray) -> np.ndarray:
    global LAST_EXEC_NS, LAST_PROFILE
    from concourse import bass_utils

    image_embeds = np.ascontiguousarray(np.asarray(image_embeds, dtype=np.float32))
    text_embeds = np.ascontiguousarray(np.asarray(text_embeds, dtype=np.float32))
    assert image_embeds.shape == (N_TOTAL, D_FULL)
    assert text_embeds.shape == (N_TOTAL, D_FULL)

    nc = _get_nc()
    blk = N_TOTAL // N_CORES
    in_maps = [
        {
            "img_block": np.ascontiguousarray(image_embeds[c * blk : (c + 1) * blk]),
            "txt_block": np.ascontiguousarray(text_embeds[c * blk : (c + 1) * blk]),
        }
        for c in range(N_CORES)
    ]
    trace = os.environ.get("KERNEL_TRACE", "0") == "1"
    res = bass_utils.run_bass_kernel_spmd(
        nc, in_maps, core_ids=list(range(N_CORES)), trace=trace
    )
    LAST_EXEC_NS = res.exec_time_ns
    LAST_PROFILE = res.profile_json
    globals()["LAST_RESULT"] = res

    mt = blk // P
    rows = []
    for c in range(N_CORES):
        o = np.asarray(res.results[c]["out_rows"])  # [P, mt]
        rows.append(o.T.reshape(-1))  # local row i = m*P + p
    vals = np.concatenate(rows)  # [N_TOTAL]
    result = np.float32(np.mean(vals.astype(np.float64)) - MARGIN)
    return np.asarray(result, dtype=np.float32)
